# revision 2
# baseline (speedup 1.0000x reference)
"""Trainium2 kernel for nn_GTLayer (GTLayer first=True forward).

Computes, for edge lists of R=5 relation graphs over N=4096 nodes:
    A   = dense_adj(edge_index, edge_value)            # [R, N, N] (coalesce add)
    F1  = softmax(w1, axis=1); F2 = softmax(w2, axis=1)
    A1  = einsum('cr,rmn->cmn', F1, A); A2 = ... F2 ...
    H   = A1 @ A2 per channel, thresholded at 0.05
    returns (H, F1, F2)

Sharding over 8 NeuronCores: (channel c in {0,1}) x (2x2 blocks of H).
Core (c, qm, qn) computes H[c][qm*2048:(qm+1)*2048, qn*2048:(qn+1)*2048].

Host side only coalesces/partitions the edge lists (index bookkeeping):
the weighted dense adjacency blocks are BUILT ON DEVICE by gpsimd
local_scatter (f32 values scattered as u16 pairs), and the per-channel
spspmm runs as fp32 PE matmuls with PSUM k-accumulation; the >0.05
threshold fuses into PSUM evacuation on the vector engine.
"""

import sys

sys.path.insert(0, "/opt/trn_rl_repo")

import numpy as np
import concourse.mybir as mybir
import concourse.tile as tile
from concourse import bass_utils, bacc, library_config

THR = 0.05
N = 4096
HALF = N // 2
PT = N // 128      # partition tiles per [N, HALF] matrix
CELL = 512         # f32 columns per local_scatter call
CH = HALF // CELL  # scatter calls per partition-tile row
NI = 128           # u16 slots per (row, CELL) cell => max 64 f32 values
NE_U16 = CELL * 2
NPAN = 2           # A2 column panels held in SBUF during matmul
PANW = HALF // NPAN
MM_N = 512         # moving free dim per matmul (one PSUM bank, fp32)


def build_nc():
    nc = bacc.Bacc(None, target_bir_lowering=False)
    a1t_idx = nc.dram_tensor("a1t_idx", [PT, 128, CH * NI], mybir.dt.int16, kind="ExternalInput")
    a1t_dat = nc.dram_tensor("a1t_dat", [PT, 128, CH * NI], mybir.dt.uint16, kind="ExternalInput")
    a2_idx = nc.dram_tensor("a2_idx", [PT, 128, CH * NI], mybir.dt.int16, kind="ExternalInput")
    a2_dat = nc.dram_tensor("a2_dat", [PT, 128, CH * NI], mybir.dt.uint16, kind="ExternalInput")
    h = nc.dram_tensor("h", [HALF, HALF], mybir.dt.float32, kind="ExternalOutput")

    with tile.TileContext(nc) as tc:
        with (
            tc.tile_pool(name="dram", bufs=1, space="DRAM") as dpool,
            tc.tile_pool(name="bld", bufs=3) as bpool,
            tc.tile_pool(name="mm", bufs=1) as mpool,
            tc.tile_pool(name="mcol", bufs=2) as cpool,
            tc.tile_pool(name="st", bufs=2) as spool,
            tc.tile_pool(name="psum", bufs=2, space="PSUM") as ppool,
        ):
            nc.gpsimd.load_library(library_config.local_scatter)

            a1t_dense = dpool.tile([PT, 128, HALF], mybir.dt.float32, tag="a1d")
            a2_dense = dpool.tile([PT, 128, HALF], mybir.dt.float32, tag="a2d")

            # phase 1: dense builds (A2 first; matmul consumes its panels first)
            for idx_t, dat_t, dense_t in (
                (a2_idx, a2_dat, a2_dense),
                (a1t_idx, a1t_dat, a1t_dense),
            ):
                for p in range(PT):
                    it = bpool.tile([128, CH * NI], mybir.dt.int16, tag="bi")
                    dt_ = bpool.tile([128, CH * NI], mybir.dt.uint16, tag="bd")
                    nc.sync.dma_start(out=it[:], in_=idx_t[p, :, :])
                    nc.sync.dma_start(out=dt_[:], in_=dat_t[p, :, :])
                    dense = bpool.tile([128, HALF], mybir.dt.float32, tag="bo")
                    dense_u16 = dense[:].bitcast(mybir.dt.uint16)
                    for c in range(CH):
                        nc.gpsimd.local_scatter(
                            out_ap=dense_u16[:, c * NE_U16 : (c + 1) * NE_U16],
                            data_ap=dt_[:, c * NI : (c + 1) * NI],
                            idxs_ap=it[:, c * NI : (c + 1) * NI],
                            channels=128,
                            num_elems=NE_U16,
                            num_idxs=NI,
                        )
                    nc.sync.dma_start(out=dense_t[p, :, :], in_=dense[:])

            # phase 2: H block = A1row @ A2col, threshold on PSUM evacuation
            for pan in range(NPAN):
                a2p = mpool.tile([128, PT, PANW], mybir.dt.float32, tag="a2p")
                nc.sync.dma_start(
                    out=a2p[:],
                    in_=a2_dense[:, :, pan * PANW : (pan + 1) * PANW].rearrange(
                        "k p n -> p k n"
                    ),
                )
                for m in range(HALF // 128):
                    a1c = cpool.tile([128, PT, 128], mybir.dt.float32, tag="a1c")
                    nc.sync.dma_start(
                        out=a1c[:],
                        in_=a1t_dense[:, :, m * 128 : (m + 1) * 128].rearrange(
                            "k p mm -> p k mm"
                        ),
                    )
                    stage = spool.tile([128, PANW], mybir.dt.float32, tag="hs")
                    for ns in range(PANW // MM_N):
                        psum = ppool.tile([128, MM_N], mybir.dt.float32, tag="ps")
                        for kt in range(PT):
                            nc.tensor.matmul(
                                psum[:],
                                a1c[:, kt, :],
                                a2p[:, kt, ns * MM_N : (ns + 1) * MM_N],
                                start=(kt == 0),
                                stop=(kt == PT - 1),
                            )
                        mask = spool.tile([128, MM_N], mybir.dt.float32, tag="msk")
                        nc.vector.tensor_scalar(
                            out=mask[:],
                            in0=psum[:],
                            scalar1=float(THR),
                            scalar2=None,
                            op0=mybir.AluOpType.is_gt,
                        )
                        nc.vector.tensor_tensor(
                            out=stage[:, ns * MM_N : (ns + 1) * MM_N],
                            in0=psum[:],
                            in1=mask[:],
                            op=mybir.AluOpType.mult,
                        )
                    nc.sync.dma_start(
                        out=h[m * 128 : (m + 1) * 128, pan * PANW : (pan + 1) * PANW],
                        in_=stage[:],
                    )
    nc.compile()
    return nc


def _softmax64(w):
    w = np.asarray(w, np.float64)
    e = np.exp(w - w.max(axis=1, keepdims=True))
    return e / e.sum(axis=1, keepdims=True)


def _host_prepare(edge_index, edge_value, w1, w2):
    """Coalesce edge lists and partition them into per-core staged scatter
    inputs (pure index bookkeeping + per-edge weight scaling)."""
    ei = np.asarray(edge_index).astype(np.int64)
    ev = np.asarray(edge_value).astype(np.float64)
    r_, _, e_ = ei.shape
    src = ei[:, 0, :].ravel()
    dst = ei[:, 1, :].ravel()
    vals = ev.reshape(-1)

    F1 = _softmax64(w1)
    F2 = _softmax64(w2)

    pos = src * N + dst
    uniq, inv = np.unique(pos, return_inverse=True)
    U = uniq.size
    rel = np.repeat(np.arange(r_), e_)
    v1 = np.stack(
        [np.bincount(inv, weights=vals * F1[c][rel], minlength=U) for c in range(2)]
    ).astype(np.float32)
    v2 = np.stack(
        [np.bincount(inv, weights=vals * F2[c][rel], minlength=U) for c in range(2)]
    ).astype(np.float32)
    ui = (uniq // N).astype(np.int64)
    uj = (uniq % N).astype(np.int64)

    def layout(part, colf, vlist):
        ptile = part >> 7
        p = part & 127
        ck = colf // CELL
        within = colf % CELL
        cellid = (ptile * 128 + p) * CH + ck
        order = np.argsort(cellid, kind="stable")
        cs = cellid[order]
        first = np.r_[True, cs[1:] != cs[:-1]] if cs.size else np.array([], bool)
        idx_first = np.maximum.accumulate(np.where(first, np.arange(cs.size), 0))
        rank = np.arange(cs.size) - idx_first
        assert cs.size == 0 or rank.max() < NI // 2, f"cell overflow: {rank.max()}"
        pt_o = ptile[order]
        p_o = p[order]
        base = ck[order] * NI + 2 * rank
        w2_ = (within[order] * 2).astype(np.int16)
        idx_arr = np.full((PT, 128, CH * NI), -1, np.int16)
        idx_arr[pt_o, p_o, base] = w2_
        idx_arr[pt_o, p_o, base + 1] = w2_ + 1
        dats = []
        for v in vlist:
            vb = np.ascontiguousarray(v[order]).view(np.uint32)
            d = np.zeros((PT, 128, CH * NI), np.uint16)
            d[pt_o, p_o, base] = (vb & 0xFFFF).astype(np.uint16)
            d[pt_o, p_o, base + 1] = (vb >> 16).astype(np.uint16)
            dats.append(d)
        return idx_arr, dats

    a2_staged = {}
    a1t_staged = {}
    for q in range(2):
        m2 = (uj // HALF) == q
        a2_staged[q] = layout(ui[m2], uj[m2] - q * HALF, [v2[0][m2], v2[1][m2]])
        m1 = (ui // HALF) == q
        a1t_staged[q] = layout(uj[m1], ui[m1] - q * HALF, [v1[0][m1], v1[1][m1]])

    in_maps = []
    for c in range(2):
        for qm in range(2):
            for qn in range(2):
                i1, d1 = a1t_staged[qm]
                i2, d2 = a2_staged[qn]
                in_maps.append(
                    {"a1t_idx": i1, "a1t_dat": d1[c], "a2_idx": i2, "a2_dat": d2[c]}
                )
    return in_maps, F1.astype(np.float32), F2.astype(np.float32)


_NC_CACHE = {}
LAST_EXEC_TIME_NS = None


def kernel(edge_index, edge_value, w1, w2, num_nodes=None, trace=False, tmpdir=None):
    """Full-input entry point: returns (H [2,4096,4096] f32, F1 [2,5], F2 [2,5])."""
    global LAST_EXEC_TIME_NS
    in_maps, F1, F2 = _host_prepare(edge_index, edge_value, w1, w2)
    if "nc" not in _NC_CACHE:
        _NC_CACHE["nc"] = build_nc()
    nc = _NC_CACHE["nc"]
    if trace:
        from antenv import axon_hooks

        axon_hooks.install_default_hook()
    res = bass_utils.run_bass_kernel_spmd(
        nc, in_maps, core_ids=list(range(8)), trace=trace, tmpdir=tmpdir
    )
    LAST_EXEC_TIME_NS = res.exec_time_ns
    H = np.empty((2, N, N), np.float32)
    for c in range(2):
        for qm in range(2):
            for qn in range(2):
                blk = res.results[c * 4 + qm * 2 + qn]["h"]
                H[c, qm * HALF : (qm + 1) * HALF, qn * HALF : (qn + 1) * HALF] = blk
    return H, F1, F2


# revision 3
# speedup vs baseline: 1.8373x; 1.8373x over previous
"""Trainium2 Bass kernel for nn_GTLayer (GTLayer first=True forward).

Reference computation (N=4096 nodes, R=5 relations, C=2 channels):
    A   = dense_adj(edge_index, edge_value)        # [R, N, N], coalesce-add
    F1  = softmax(w1, axis=1); F2 = softmax(w2, axis=1)
    A1  = einsum('cr,rmn->cmn', F1, A); A2 = einsum('cr,rmn->cmn', F2, A)
    H   = (A1 @ A2 per channel) thresholded at > 0.05
    returns (H, F1, F2)

Sharding over 8 NeuronCores: (channel c in {0,1}) x (2x2 blocks of H).
Core (c, qm, qn) computes H[c][qm*2048:(qm+1)*2048, qn*2048:(qn+1)*2048].

Host does only edge-list bookkeeping (coalescing duplicates, weighting by
the softmaxed channel mix, partitioning into per-(partition-row, 512-col)
cell lists).  On device, each core:
  - scatter-builds its dense A2 column panels DIRECTLY IN SBUF and its
    dense A1^T half (chunk-major) into DRAM scratch, using gpsimd
    local_scatter (f32 values moved exactly as u16 pairs);
  - runs the per-channel spspmm as dense PE matmuls (float32r single-pass,
    fp32 PSUM accumulation over the 4096-deep contraction);
  - fuses the >0.05 threshold into PSUM evacuation on the vector engine.

Set GT_KERNEL_MM_DTYPE=fp32 in the environment for exact-fp32 matmuls
(~2x slower, rel err ~2e-4 instead of ~5e-3).
"""

import os
import sys

sys.path.insert(0, "/opt/trn_rl_repo")

import numpy as np
import concourse.mybir as mybir
import concourse.tile as tile
from concourse import bass_utils, bacc, library_config

THR = 0.05
N = 4096
HALF = N // 2
PT = N // 128      # 128-row partition tiles per [N, HALF] matrix
CELL = 512         # f32 columns per local_scatter call (= A2 panel width)
CH = HALF // CELL  # scatter calls per partition-tile row / # panels
NI = 128           # u16 slots per (row, CELL) cell => max 64 f32 values
NE_U16 = CELL * 2
MM_DTYPE = os.environ.get("GT_KERNEL_MM_DTYPE", "f32r")


def build_nc(mm_dtype=MM_DTYPE):
    cell, ni, pt, ch = CELL, NI, PT, CH
    panw, npan = CELL, CH
    nc = bacc.Bacc(None, target_bir_lowering=False)
    a1t_idx = nc.dram_tensor("a1t_idx", [pt, 128, ch * ni], mybir.dt.int16, kind="ExternalInput")
    a1t_dat = nc.dram_tensor("a1t_dat", [pt, 128, ch * ni], mybir.dt.uint16, kind="ExternalInput")
    a2_idx = nc.dram_tensor("a2_idx", [pt, 128, ch * ni], mybir.dt.int16, kind="ExternalInput")
    a2_dat = nc.dram_tensor("a2_dat", [pt, 128, ch * ni], mybir.dt.uint16, kind="ExternalInput")
    h = nc.dram_tensor("h", [HALF, HALF], mybir.dt.float32, kind="ExternalOutput")

    with tile.TileContext(nc) as tc:
        with (
            tc.tile_pool(name="dram", bufs=1, space="DRAM") as dpool,
            tc.tile_pool(name="stg", bufs=1) as gpool,
            tc.tile_pool(name="bld", bufs=2) as bpool,
            tc.tile_pool(name="pan", bufs=2) as mpool,
            tc.tile_pool(name="mcol", bufs=2) as cpool,
            tc.tile_pool(name="st", bufs=2) as spool,
            tc.tile_pool(name="psum", bufs=2, space="PSUM") as ppool,
        ):
            nc.gpsimd.load_library(library_config.local_scatter)

            a1t_dt = mybir.dt.float32r if mm_dtype == "f32r" else mybir.dt.float32
            a1t_dense = dpool.tile([pt, 128, HALF], a1t_dt, tag="a1d")

            def stage_in(idx_t, dat_t, sl):
                it = gpool.tile([128, pt, ni], mybir.dt.int16, tag="si")
                dt_ = gpool.tile([128, pt, ni], mybir.dt.uint16, tag="sd")
                nc.sync.dma_start(out=it[:], in_=idx_t[:, :, sl].rearrange("t p s -> p t s"))
                nc.sync.dma_start(out=dt_[:], in_=dat_t[:, :, sl].rearrange("t p s -> p t s"))
                return it, dt_

            def build_a1t_chunk(ck):
                sl = slice(ck * ni, (ck + 1) * ni)
                it, dt_ = stage_in(a1t_idx, a1t_dat, sl)
                for p in range(pt):
                    dense = bpool.tile([128, cell], mybir.dt.float32, tag="bo")
                    nc.gpsimd.local_scatter(
                        out_ap=dense[:].bitcast(mybir.dt.uint16),
                        data_ap=dt_[:, p, :],
                        idxs_ap=it[:, p, :],
                        channels=128,
                        num_elems=NE_U16,
                        num_idxs=ni,
                    )
                    if mm_dtype == "f32r":
                        dr = bpool.tile([128, cell], mybir.dt.float32r, tag="br")
                        nc.vector.tensor_copy(out=dr[:], in_=dense[:])
                        dense = dr
                    nc.sync.dma_start(
                        out=a1t_dense[p, :, ck * cell : (ck + 1) * cell], in_=dense[:]
                    )

            def build_a2_panel(pan):
                sl = slice(pan * ni, (pan + 1) * ni)
                it, dt_ = stage_in(a2_idx, a2_dat, sl)
                if mm_dtype == "f32r":
                    # scatter into a small f32 tile and DVE-round into the
                    # f32r panel (the BIR verifier requires every matmul input
                    # to be written by an f32r-producing instruction)
                    a2p = mpool.tile([128, pt, panw], mybir.dt.float32r, tag="a2p")
                    for kt in range(pt):
                        sm = bpool.tile([128, panw], mybir.dt.float32, tag="a2s")
                        nc.gpsimd.local_scatter(
                            out_ap=sm[:].bitcast(mybir.dt.uint16),
                            data_ap=dt_[:, kt, :],
                            idxs_ap=it[:, kt, :],
                            channels=128,
                            num_elems=NE_U16,
                            num_idxs=ni,
                        )
                        nc.vector.tensor_copy(out=a2p[:, kt, :], in_=sm[:])
                    return a2p
                a2p = mpool.tile([128, pt, panw], mybir.dt.float32, tag="a2p")
                a2p_u16 = a2p[:].bitcast(mybir.dt.uint16)
                for kt in range(pt):
                    nc.gpsimd.local_scatter(
                        out_ap=a2p_u16[:, kt, :],
                        data_ap=dt_[:, kt, :],
                        idxs_ap=it[:, kt, :],
                        channels=128,
                        num_elems=NE_U16,
                        num_idxs=ni,
                    )
                return a2p

            # gpsimd emission order: panel 0 + ALL A1T chunks first (the PE
            # consumes A1T chunks faster than the builder early on).  Panels
            # 1+ carry WAR waits on panel-buffer reuse, so they must come
            # after every A1T chunk or build/matmul dependencies cycle.
            panels = [build_a2_panel(0)]
            for ck in range(ch):
                build_a1t_chunk(ck)
            for pan in range(1, npan):
                panels.append(build_a2_panel(pan))

            for pan in range(npan):
                a2p = panels[pan]
                for m in range(HALF // 128):
                    a1c = cpool.tile([128, pt, 128], a1t_dt, tag="a1c")
                    nc.sync.dma_start(
                        out=a1c[:],
                        in_=a1t_dense[:, :, m * 128 : (m + 1) * 128].rearrange(
                            "k p mm -> p k mm"
                        ),
                    )
                    psum = ppool.tile([128, panw], mybir.dt.float32, tag="ps")
                    for kt in range(pt):
                        nc.tensor.matmul(
                            psum[:],
                            a1c[:, kt, :],
                            a2p[:, kt, :],
                            start=(kt == 0),
                            stop=(kt == pt - 1),
                        )
                    # threshold: stage = (psum > THR) * psum; mask lands in
                    # stage, then multiplied in place (one PSUM read per op)
                    stage = spool.tile([128, panw], mybir.dt.float32, tag="hs")
                    nc.vector.tensor_scalar(
                        out=stage[:], in0=psum[:], scalar1=float(THR), scalar2=None,
                        op0=mybir.AluOpType.is_gt,
                    )
                    nc.vector.tensor_tensor(
                        out=stage[:], in0=psum[:], in1=stage[:], op=mybir.AluOpType.mult,
                    )
                    nc.sync.dma_start(
                        out=h[m * 128 : (m + 1) * 128, pan * panw : (pan + 1) * panw],
                        in_=stage[:],
                    )
    nc.compile()
    return nc


def _softmax64(w):
    w = np.asarray(w, np.float64)
    e = np.exp(w - w.max(axis=1, keepdims=True))
    return e / e.sum(axis=1, keepdims=True)


def _host_prepare(edge_index, edge_value, w1, w2):
    """Coalesce the edge lists and partition them into per-core staged
    scatter inputs (index bookkeeping + per-edge channel weighting only)."""
    ei = np.asarray(edge_index).astype(np.int64)
    ev = np.asarray(edge_value).astype(np.float64)
    r_, _, e_ = ei.shape
    src = ei[:, 0, :].ravel()
    dst = ei[:, 1, :].ravel()
    vals = ev.reshape(-1)

    F1 = _softmax64(w1)
    F2 = _softmax64(w2)

    pos = src * N + dst
    uniq, inv = np.unique(pos, return_inverse=True)
    U = uniq.size
    rel = np.repeat(np.arange(r_), e_)
    v1 = np.stack(
        [np.bincount(inv, weights=vals * F1[c][rel], minlength=U) for c in range(2)]
    ).astype(np.float32)
    v2 = np.stack(
        [np.bincount(inv, weights=vals * F2[c][rel], minlength=U) for c in range(2)]
    ).astype(np.float32)
    ui = (uniq // N).astype(np.int64)
    uj = (uniq % N).astype(np.int64)

    def layout(part, colf, vlist):
        ptile = part >> 7
        p = part & 127
        ck = colf // CELL
        within = colf % CELL
        cellid = (ptile * 128 + p) * CH + ck
        order = np.argsort(cellid, kind="stable")
        cs = cellid[order]
        first = np.r_[True, cs[1:] != cs[:-1]] if cs.size else np.array([], bool)
        idx_first = np.maximum.accumulate(np.where(first, np.arange(cs.size), 0))
        rank = np.arange(cs.size) - idx_first
        assert cs.size == 0 or rank.max() < NI // 2, f"cell overflow: {rank.max()}"
        pt_o = ptile[order]
        p_o = p[order]
        base = ck[order] * NI + 2 * rank
        w2_ = (within[order] * 2).astype(np.int16)
        idx_arr = np.full((PT, 128, CH * NI), -1, np.int16)
        idx_arr[pt_o, p_o, base] = w2_
        idx_arr[pt_o, p_o, base + 1] = w2_ + 1
        dats = []
        for v in vlist:
            vb = np.ascontiguousarray(v[order]).view(np.uint32)
            d = np.zeros((PT, 128, CH * NI), np.uint16)
            d[pt_o, p_o, base] = (vb & 0xFFFF).astype(np.uint16)
            d[pt_o, p_o, base + 1] = (vb >> 16).astype(np.uint16)
            dats.append(d)
        return idx_arr, dats

    # A2[qn-half]: entry A[i, j] at partition i, col j - qn*HALF
    # A1T[qm-half]: entry A[i, j] at partition j, col i - qm*HALF
    a2_staged = {}
    a1t_staged = {}
    for q in range(2):
        m2 = (uj // HALF) == q
        a2_staged[q] = layout(ui[m2], uj[m2] - q * HALF, [v2[0][m2], v2[1][m2]])
        m1 = (ui // HALF) == q
        a1t_staged[q] = layout(uj[m1], ui[m1] - q * HALF, [v1[0][m1], v1[1][m1]])

    in_maps = []
    for c in range(2):
        for qm in range(2):
            for qn in range(2):
                i1, d1 = a1t_staged[qm]
                i2, d2 = a2_staged[qn]
                in_maps.append(
                    {"a1t_idx": i1, "a1t_dat": d1[c], "a2_idx": i2, "a2_dat": d2[c]}
                )
    return in_maps, F1.astype(np.float32), F2.astype(np.float32)


_NC_CACHE = {}
LAST_EXEC_TIME_NS = None


def _run_with_retry(nc, in_maps, trace, tmpdir):
    """First execution of a freshly compiled NEFF occasionally dies with
    NRT_EXEC_UNIT_UNRECOVERABLE on this setup; a re-execution succeeds."""
    last = None
    for attempt in range(3):
        try:
            return bass_utils.run_bass_kernel_spmd(
                nc, in_maps, core_ids=list(range(8)), trace=trace, tmpdir=tmpdir
            )
        except Exception as e:  # noqa: BLE001
            last = e
            sys.stderr.write(
                f"kernel: device run attempt {attempt} failed "
                f"({type(e).__name__}); retrying\n"
            )
    raise last


def kernel(edge_index, edge_value, w1, w2, num_nodes=None, trace=False, tmpdir=None):
    """Full-input entry: returns (H [2,4096,4096] f32, F1 [2,5] f32, F2 [2,5] f32)."""
    global LAST_EXEC_TIME_NS
    in_maps, F1, F2 = _host_prepare(edge_index, edge_value, w1, w2)
    if MM_DTYPE not in _NC_CACHE:
        _NC_CACHE[MM_DTYPE] = build_nc(MM_DTYPE)
    nc = _NC_CACHE[MM_DTYPE]
    if trace:
        from antenv import axon_hooks

        axon_hooks.install_default_hook()
    res = _run_with_retry(nc, in_maps, trace, tmpdir)
    LAST_EXEC_TIME_NS = res.exec_time_ns
    H = np.empty((2, N, N), np.float32)
    for c in range(2):
        for qm in range(2):
            for qn in range(2):
                blk = res.results[c * 4 + qm * 2 + qn]["h"]
                H[c, qm * HALF : (qm + 1) * HALF, qn * HALF : (qn + 1) * HALF] = blk
    return H, F1, F2


# revision 4
# speedup vs baseline: 2.3591x; 1.2840x over previous
"""Trainium2 Bass kernel for nn_GTLayer (GTLayer first=True forward).

Reference computation (N=4096 nodes, R=5 relations, C=2 channels):
    A   = dense_adj(edge_index, edge_value)        # [R, N, N], coalesce-add
    F1  = softmax(w1, axis=1); F2 = softmax(w2, axis=1)
    A1  = einsum('cr,rmn->cmn', F1, A); A2 = einsum('cr,rmn->cmn', F2, A)
    H   = (A1 @ A2 per channel) thresholded at > 0.05
    returns (H, F1, F2)

Sharding over 8 NeuronCores: (channel c in {0,1}) x (2x2 blocks of H).
Core (c, qm, qn) computes H[c][qm*2048:(qm+1)*2048, qn*2048:(qn+1)*2048].

Host does only edge-list bookkeeping (coalescing duplicates, weighting by
the softmaxed channel mix, partitioning into per-(partition-row, 512-col)
cell lists).  On device, each core:
  - scatter-builds its dense A2 column panels DIRECTLY IN SBUF and its
    dense A1^T half (chunk-major) into DRAM scratch, using gpsimd
    local_scatter (f32 values moved exactly as u16 pairs);
  - runs the per-channel spspmm as dense PE matmuls (float32r single-pass,
    fp32 PSUM accumulation over the 4096-deep contraction);
  - fuses the >0.05 threshold into PSUM evacuation on the vector engine.

Set GT_KERNEL_MM_DTYPE=fp32 in the environment for exact-fp32 matmuls
(~2x slower, rel err ~2e-4 instead of ~5e-3).
"""

import os
import sys

sys.path.insert(0, "/opt/trn_rl_repo")

import numpy as np
import concourse.mybir as mybir
import concourse.tile as tile
from concourse import bass_utils, bacc, library_config

THR = 0.05
N = 4096
HALF = N // 2
PT = N // 128      # 128-row partition tiles per [N, HALF] matrix
CELL = 512         # f32 columns per local_scatter call (= A2 panel width)
CH = HALF // CELL  # scatter calls per partition-tile row / # panels
NI = 128           # u16 slots per (row, CELL) cell => max 64 f32 values
NE_U16 = CELL * 2
MM_DTYPE = os.environ.get("GT_KERNEL_MM_DTYPE", "f32r")


def build_nc(mm_dtype=MM_DTYPE):
    cell, ni, pt, ch = CELL, NI, PT, CH
    panw, npan = CELL, CH
    nc = bacc.Bacc(None, target_bir_lowering=False)
    a1t_idx = nc.dram_tensor("a1t_idx", [pt, 128, ch * ni], mybir.dt.int16, kind="ExternalInput")
    a1t_dat = nc.dram_tensor("a1t_dat", [pt, 128, ch * ni], mybir.dt.uint16, kind="ExternalInput")
    a2_idx = nc.dram_tensor("a2_idx", [pt, 128, ch * ni], mybir.dt.int16, kind="ExternalInput")
    a2_dat = nc.dram_tensor("a2_dat", [pt, 128, ch * ni], mybir.dt.uint16, kind="ExternalInput")
    h = nc.dram_tensor("h", [HALF, HALF], mybir.dt.float32, kind="ExternalOutput")

    with tile.TileContext(nc) as tc:
        with (
            tc.tile_pool(name="dram", bufs=1, space="DRAM") as dpool,
            tc.tile_pool(name="stg", bufs=1) as gpool,
            tc.tile_pool(name="bld", bufs=2) as bpool,
            tc.tile_pool(name="pan", bufs=2) as mpool,
            tc.tile_pool(name="mcol", bufs=2) as cpool,
            tc.tile_pool(name="st", bufs=2) as spool,
            tc.tile_pool(name="psum", bufs=2, space="PSUM") as ppool,
        ):
            nc.gpsimd.load_library(library_config.local_scatter)

            a1t_dt = mybir.dt.float32r if mm_dtype == "f32r" else mybir.dt.float32
            a1t_dense = dpool.tile([HALF // 128, 128, pt, 128], a1t_dt, tag="a1d")

            def stage_in(idx_t, dat_t, sl):
                it = gpool.tile([128, pt, ni], mybir.dt.int16, tag="si")
                dt_ = gpool.tile([128, pt, ni], mybir.dt.uint16, tag="sd")
                nc.sync.dma_start(out=it[:], in_=idx_t[:, :, sl].rearrange("t p s -> p t s"))
                nc.sync.dma_start(out=dt_[:], in_=dat_t[:, :, sl].rearrange("t p s -> p t s"))
                return it, dt_

            def build_a1t_chunk(ck):
                sl = slice(ck * ni, (ck + 1) * ni)
                it, dt_ = stage_in(a1t_idx, a1t_dat, sl)
                for p in range(pt):
                    dense = bpool.tile([128, cell], mybir.dt.float32, tag="bo")
                    nc.gpsimd.local_scatter(
                        out_ap=dense[:].bitcast(mybir.dt.uint16),
                        data_ap=dt_[:, p, :],
                        idxs_ap=it[:, p, :],
                        channels=128,
                        num_elems=NE_U16,
                        num_idxs=ni,
                    )
                    if mm_dtype == "f32r":
                        dr = bpool.tile([128, cell], mybir.dt.float32r, tag="br")
                        nc.vector.tensor_copy(out=dr[:], in_=dense[:])
                        dense = dr
                    # m-col-major layout: loads below are per-partition
                    # contiguous 16KB reads instead of 512B-grain gathers
                    nc.sync.dma_start(
                        out=a1t_dense[
                            ck * (cell // 128) : (ck + 1) * (cell // 128), :, p, :
                        ].rearrange("mc q mm -> q mc mm"),
                        in_=dense[:].rearrange("q (mc mm) -> q mc mm", mm=128),
                    )

            def build_a2_panel(pan):
                sl = slice(pan * ni, (pan + 1) * ni)
                it, dt_ = stage_in(a2_idx, a2_dat, sl)
                if mm_dtype == "f32r":
                    # scatter into a small f32 tile and DVE-round into the
                    # f32r panel (the BIR verifier requires every matmul input
                    # to be written by an f32r-producing instruction)
                    a2p = mpool.tile([128, pt, panw], mybir.dt.float32r, tag="a2p")
                    for kt in range(pt):
                        sm = bpool.tile([128, panw], mybir.dt.float32, tag="a2s")
                        nc.gpsimd.local_scatter(
                            out_ap=sm[:].bitcast(mybir.dt.uint16),
                            data_ap=dt_[:, kt, :],
                            idxs_ap=it[:, kt, :],
                            channels=128,
                            num_elems=NE_U16,
                            num_idxs=ni,
                        )
                        nc.vector.tensor_copy(out=a2p[:, kt, :], in_=sm[:])
                    return a2p
                a2p = mpool.tile([128, pt, panw], mybir.dt.float32, tag="a2p")
                a2p_u16 = a2p[:].bitcast(mybir.dt.uint16)
                for kt in range(pt):
                    nc.gpsimd.local_scatter(
                        out_ap=a2p_u16[:, kt, :],
                        data_ap=dt_[:, kt, :],
                        idxs_ap=it[:, kt, :],
                        channels=128,
                        num_elems=NE_U16,
                        num_idxs=ni,
                    )
                return a2p

            # gpsimd emission order: panel 0 + ALL A1T chunks first (the PE
            # consumes A1T chunks faster than the builder early on).  Panels
            # 1+ carry WAR waits on panel-buffer reuse, so they must come
            # after every A1T chunk or build/matmul dependencies cycle.
            panels = [build_a2_panel(0)]
            for ck in range(ch):
                build_a1t_chunk(ck)
            for pan in range(1, npan):
                panels.append(build_a2_panel(pan))

            for pan in range(npan):
                a2p = panels[pan]
                for m in range(HALF // 128):
                    a1c = cpool.tile([128, pt, 128], a1t_dt, tag="a1c")
                    nc.sync.dma_start(out=a1c[:], in_=a1t_dense[m, :, :, :])
                    psum = ppool.tile([128, panw], mybir.dt.float32, tag="ps")
                    for kt in range(pt):
                        nc.tensor.matmul(
                            psum[:],
                            a1c[:, kt, :],
                            a2p[:, kt, :],
                            start=(kt == 0),
                            stop=(kt == pt - 1),
                        )
                    # threshold: stage = (psum > THR) * psum; mask lands in
                    # stage, then multiplied in place (one PSUM read per op)
                    stage = spool.tile([128, panw], mybir.dt.float32, tag="hs")
                    nc.vector.tensor_scalar(
                        out=stage[:], in0=psum[:], scalar1=float(THR), scalar2=None,
                        op0=mybir.AluOpType.is_gt,
                    )
                    nc.vector.tensor_tensor(
                        out=stage[:], in0=psum[:], in1=stage[:], op=mybir.AluOpType.mult,
                    )
                    nc.sync.dma_start(
                        out=h[m * 128 : (m + 1) * 128, pan * panw : (pan + 1) * panw],
                        in_=stage[:],
                    )
    nc.compile()
    return nc


def _softmax64(w):
    w = np.asarray(w, np.float64)
    e = np.exp(w - w.max(axis=1, keepdims=True))
    return e / e.sum(axis=1, keepdims=True)


def _host_prepare(edge_index, edge_value, w1, w2):
    """Coalesce the edge lists and partition them into per-core staged
    scatter inputs (index bookkeeping + per-edge channel weighting only)."""
    ei = np.asarray(edge_index).astype(np.int64)
    ev = np.asarray(edge_value).astype(np.float64)
    r_, _, e_ = ei.shape
    src = ei[:, 0, :].ravel()
    dst = ei[:, 1, :].ravel()
    vals = ev.reshape(-1)

    F1 = _softmax64(w1)
    F2 = _softmax64(w2)

    pos = src * N + dst
    uniq, inv = np.unique(pos, return_inverse=True)
    U = uniq.size
    rel = np.repeat(np.arange(r_), e_)
    v1 = np.stack(
        [np.bincount(inv, weights=vals * F1[c][rel], minlength=U) for c in range(2)]
    ).astype(np.float32)
    v2 = np.stack(
        [np.bincount(inv, weights=vals * F2[c][rel], minlength=U) for c in range(2)]
    ).astype(np.float32)
    ui = (uniq // N).astype(np.int64)
    uj = (uniq % N).astype(np.int64)

    def layout(part, colf, vlist):
        ptile = part >> 7
        p = part & 127
        ck = colf // CELL
        within = colf % CELL
        cellid = (ptile * 128 + p) * CH + ck
        order = np.argsort(cellid, kind="stable")
        cs = cellid[order]
        first = np.r_[True, cs[1:] != cs[:-1]] if cs.size else np.array([], bool)
        idx_first = np.maximum.accumulate(np.where(first, np.arange(cs.size), 0))
        rank = np.arange(cs.size) - idx_first
        assert cs.size == 0 or rank.max() < NI // 2, f"cell overflow: {rank.max()}"
        pt_o = ptile[order]
        p_o = p[order]
        base = ck[order] * NI + 2 * rank
        w2_ = (within[order] * 2).astype(np.int16)
        idx_arr = np.full((PT, 128, CH * NI), -1, np.int16)
        idx_arr[pt_o, p_o, base] = w2_
        idx_arr[pt_o, p_o, base + 1] = w2_ + 1
        dats = []
        for v in vlist:
            vb = np.ascontiguousarray(v[order]).view(np.uint32)
            d = np.zeros((PT, 128, CH * NI), np.uint16)
            d[pt_o, p_o, base] = (vb & 0xFFFF).astype(np.uint16)
            d[pt_o, p_o, base + 1] = (vb >> 16).astype(np.uint16)
            dats.append(d)
        return idx_arr, dats

    # A2[qn-half]: entry A[i, j] at partition i, col j - qn*HALF
    # A1T[qm-half]: entry A[i, j] at partition j, col i - qm*HALF
    a2_staged = {}
    a1t_staged = {}
    for q in range(2):
        m2 = (uj // HALF) == q
        a2_staged[q] = layout(ui[m2], uj[m2] - q * HALF, [v2[0][m2], v2[1][m2]])
        m1 = (ui // HALF) == q
        a1t_staged[q] = layout(uj[m1], ui[m1] - q * HALF, [v1[0][m1], v1[1][m1]])

    in_maps = []
    for c in range(2):
        for qm in range(2):
            for qn in range(2):
                i1, d1 = a1t_staged[qm]
                i2, d2 = a2_staged[qn]
                in_maps.append(
                    {"a1t_idx": i1, "a1t_dat": d1[c], "a2_idx": i2, "a2_dat": d2[c]}
                )
    return in_maps, F1.astype(np.float32), F2.astype(np.float32)


_NC_CACHE = {}
LAST_EXEC_TIME_NS = None


def _run_with_retry(nc, in_maps, trace, tmpdir):
    """First execution of a freshly compiled NEFF occasionally dies with
    NRT_EXEC_UNIT_UNRECOVERABLE on this setup; a re-execution succeeds."""
    last = None
    for attempt in range(3):
        try:
            return bass_utils.run_bass_kernel_spmd(
                nc, in_maps, core_ids=list(range(8)), trace=trace, tmpdir=tmpdir
            )
        except Exception as e:  # noqa: BLE001
            last = e
            sys.stderr.write(
                f"kernel: device run attempt {attempt} failed "
                f"({type(e).__name__}); retrying\n"
            )
    raise last


def kernel(edge_index, edge_value, w1, w2, num_nodes=None, trace=False, tmpdir=None):
    """Full-input entry: returns (H [2,4096,4096] f32, F1 [2,5] f32, F2 [2,5] f32)."""
    global LAST_EXEC_TIME_NS
    in_maps, F1, F2 = _host_prepare(edge_index, edge_value, w1, w2)
    if MM_DTYPE not in _NC_CACHE:
        _NC_CACHE[MM_DTYPE] = build_nc(MM_DTYPE)
    nc = _NC_CACHE[MM_DTYPE]
    if trace:
        from antenv import axon_hooks

        axon_hooks.install_default_hook()
    res = _run_with_retry(nc, in_maps, trace, tmpdir)
    LAST_EXEC_TIME_NS = res.exec_time_ns
    H = np.empty((2, N, N), np.float32)
    for c in range(2):
        for qm in range(2):
            for qn in range(2):
                blk = res.results[c * 4 + qm * 2 + qn]["h"]
                H[c, qm * HALF : (qm + 1) * HALF, qn * HALF : (qn + 1) * HALF] = blk
    return H, F1, F2


# revision 5
# speedup vs baseline: 2.3905x; 1.0133x over previous
"""Trainium2 Bass kernel for nn_GTLayer (GTLayer first=True forward).

Reference computation (N=4096 nodes, R=5 relations, C=2 channels):
    A   = dense_adj(edge_index, edge_value)        # [R, N, N], coalesce-add
    F1  = softmax(w1, axis=1); F2 = softmax(w2, axis=1)
    A1  = einsum('cr,rmn->cmn', F1, A); A2 = einsum('cr,rmn->cmn', F2, A)
    H   = (A1 @ A2 per channel) thresholded at > 0.05
    returns (H, F1, F2)

Sharding over 8 NeuronCores: (channel c in {0,1}) x (2x2 blocks of H).
Core (c, qm, qn) computes H[c][qm*2048:(qm+1)*2048, qn*2048:(qn+1)*2048].

Host does only edge-list bookkeeping (coalescing duplicates, weighting by
the softmaxed channel mix, partitioning into per-(partition-row, 512-col)
cell lists).  On device, each core:
  - scatter-builds its dense A2 column panels DIRECTLY IN SBUF and its
    dense A1^T half (chunk-major) into DRAM scratch, using gpsimd
    local_scatter (f32 values moved exactly as u16 pairs);
  - runs the per-channel spspmm as dense PE matmuls (float32r single-pass,
    fp32 PSUM accumulation over the 4096-deep contraction);
  - fuses the >0.05 threshold into PSUM evacuation on the vector engine.

Set GT_KERNEL_MM_DTYPE=fp32 in the environment for exact-fp32 matmuls
(~2x slower, rel err ~2e-4 instead of ~5e-3).
"""

import os
import sys

sys.path.insert(0, "/opt/trn_rl_repo")

import numpy as np
import concourse.mybir as mybir
import concourse.tile as tile
from concourse import bass_utils, bacc, library_config

THR = 0.05
N = 4096
HALF = N // 2
PT = N // 128      # 128-row partition tiles per [N, HALF] matrix
CELL = 512         # f32 columns per local_scatter call (= A2 panel width)
CH = HALF // CELL  # scatter calls per partition-tile row / # panels
NI = 112           # u16 slots per (row, CELL) cell (max observed occupancy 80)
NE_U16 = CELL * 2
MM_DTYPE = os.environ.get("GT_KERNEL_MM_DTYPE", "f32r")


def build_nc(mm_dtype=MM_DTYPE):
    cell, ni, pt, ch = CELL, NI, PT, CH
    panw, npan = CELL, CH
    nc = bacc.Bacc(None, target_bir_lowering=False)
    a1t_idx = nc.dram_tensor("a1t_idx", [pt, 128, ch * ni], mybir.dt.int16, kind="ExternalInput")
    a1t_dat = nc.dram_tensor("a1t_dat", [pt, 128, ch * ni], mybir.dt.uint16, kind="ExternalInput")
    a2_idx = nc.dram_tensor("a2_idx", [pt, 128, ch * ni], mybir.dt.int16, kind="ExternalInput")
    a2_dat = nc.dram_tensor("a2_dat", [pt, 128, ch * ni], mybir.dt.uint16, kind="ExternalInput")
    h = nc.dram_tensor("h", [HALF, HALF], mybir.dt.float32, kind="ExternalOutput")

    with tile.TileContext(nc) as tc:
        with (
            tc.tile_pool(name="dram", bufs=1, space="DRAM") as dpool,
            tc.tile_pool(name="stg", bufs=2) as gpool,
            tc.tile_pool(name="bld", bufs=2) as bpool,
            tc.tile_pool(name="pan", bufs=2) as mpool,
            tc.tile_pool(name="mcol", bufs=2) as cpool,
            tc.tile_pool(name="st", bufs=2) as spool,
            tc.tile_pool(name="psum", bufs=2, space="PSUM") as ppool,
        ):
            nc.gpsimd.load_library(library_config.local_scatter)

            a1t_dt = mybir.dt.float32r if mm_dtype == "f32r" else mybir.dt.float32
            a1t_dense = dpool.tile([HALF // 128, 128, pt, 128], a1t_dt, tag="a1d")

            def stage_in(idx_t, dat_t, sl):
                it = gpool.tile([128, pt, ni], mybir.dt.int16, tag="si")
                dt_ = gpool.tile([128, pt, ni], mybir.dt.uint16, tag="sd")
                nc.sync.dma_start(out=it[:], in_=idx_t[:, :, sl].rearrange("t p s -> p t s"))
                nc.sync.dma_start(out=dt_[:], in_=dat_t[:, :, sl].rearrange("t p s -> p t s"))
                return it, dt_

            def build_a1t_chunk(ck):
                sl = slice(ck * ni, (ck + 1) * ni)
                it, dt_ = stage_in(a1t_idx, a1t_dat, sl)
                for p in range(pt):
                    dense = bpool.tile([128, cell], mybir.dt.float32, tag="bo")
                    nc.gpsimd.local_scatter(
                        out_ap=dense[:].bitcast(mybir.dt.uint16),
                        data_ap=dt_[:, p, :],
                        idxs_ap=it[:, p, :],
                        channels=128,
                        num_elems=NE_U16,
                        num_idxs=ni,
                    )
                    if mm_dtype == "f32r":
                        dr = bpool.tile([128, cell], mybir.dt.float32r, tag="br")
                        nc.vector.tensor_copy(out=dr[:], in_=dense[:])
                        dense = dr
                    # m-col-major layout: loads below are per-partition
                    # contiguous 16KB reads instead of 512B-grain gathers
                    nc.sync.dma_start(
                        out=a1t_dense[
                            ck * (cell // 128) : (ck + 1) * (cell // 128), :, p, :
                        ].rearrange("mc q mm -> q mc mm"),
                        in_=dense[:].rearrange("q (mc mm) -> q mc mm", mm=128),
                    )

            def build_a2_panel(pan):
                sl = slice(pan * ni, (pan + 1) * ni)
                it, dt_ = stage_in(a2_idx, a2_dat, sl)
                if mm_dtype == "f32r":
                    # scatter into a small f32 tile and DVE-round into the
                    # f32r panel (the BIR verifier requires every matmul input
                    # to be written by an f32r-producing instruction)
                    a2p = mpool.tile([128, pt, panw], mybir.dt.float32r, tag="a2p")
                    for kt in range(pt):
                        sm = bpool.tile([128, panw], mybir.dt.float32, tag="a2s")
                        nc.gpsimd.local_scatter(
                            out_ap=sm[:].bitcast(mybir.dt.uint16),
                            data_ap=dt_[:, kt, :],
                            idxs_ap=it[:, kt, :],
                            channels=128,
                            num_elems=NE_U16,
                            num_idxs=ni,
                        )
                        nc.vector.tensor_copy(out=a2p[:, kt, :], in_=sm[:])
                    return a2p
                a2p = mpool.tile([128, pt, panw], mybir.dt.float32, tag="a2p")
                a2p_u16 = a2p[:].bitcast(mybir.dt.uint16)
                for kt in range(pt):
                    nc.gpsimd.local_scatter(
                        out_ap=a2p_u16[:, kt, :],
                        data_ap=dt_[:, kt, :],
                        idxs_ap=it[:, kt, :],
                        channels=128,
                        num_elems=NE_U16,
                        num_idxs=ni,
                    )
                return a2p

            # gpsimd emission order: panel 0 + ALL A1T chunks first (the PE
            # consumes A1T chunks faster than the builder early on).  Panels
            # 1+ carry WAR waits on panel-buffer reuse, so they must come
            # after every A1T chunk or build/matmul dependencies cycle.
            panels = [build_a2_panel(0)]
            for ck in range(ch):
                build_a1t_chunk(ck)
            for pan in range(1, npan):
                panels.append(build_a2_panel(pan))

            for pan in range(npan):
                a2p = panels[pan]
                for m in range(HALF // 128):
                    a1c = cpool.tile([128, pt, 128], a1t_dt, tag="a1c")
                    nc.sync.dma_start(out=a1c[:], in_=a1t_dense[m, :, :, :])
                    psum = ppool.tile([128, panw], mybir.dt.float32, tag="ps")
                    for kt in range(pt):
                        nc.tensor.matmul(
                            psum[:],
                            a1c[:, kt, :],
                            a2p[:, kt, :],
                            start=(kt == 0),
                            stop=(kt == pt - 1),
                        )
                    # threshold: stage = (psum > THR) * psum; mask lands in
                    # stage, then multiplied in place (one PSUM read per op)
                    stage = spool.tile([128, panw], mybir.dt.float32, tag="hs")
                    nc.vector.tensor_scalar(
                        out=stage[:], in0=psum[:], scalar1=float(THR), scalar2=None,
                        op0=mybir.AluOpType.is_gt,
                    )
                    nc.vector.tensor_tensor(
                        out=stage[:], in0=psum[:], in1=stage[:], op=mybir.AluOpType.mult,
                    )
                    nc.sync.dma_start(
                        out=h[m * 128 : (m + 1) * 128, pan * panw : (pan + 1) * panw],
                        in_=stage[:],
                    )
    nc.compile()
    return nc


def _softmax64(w):
    w = np.asarray(w, np.float64)
    e = np.exp(w - w.max(axis=1, keepdims=True))
    return e / e.sum(axis=1, keepdims=True)


def _host_prepare(edge_index, edge_value, w1, w2):
    """Coalesce the edge lists and partition them into per-core staged
    scatter inputs (index bookkeeping + per-edge channel weighting only)."""
    ei = np.asarray(edge_index).astype(np.int64)
    ev = np.asarray(edge_value).astype(np.float64)
    r_, _, e_ = ei.shape
    src = ei[:, 0, :].ravel()
    dst = ei[:, 1, :].ravel()
    vals = ev.reshape(-1)

    F1 = _softmax64(w1)
    F2 = _softmax64(w2)

    pos = src * N + dst
    uniq, inv = np.unique(pos, return_inverse=True)
    U = uniq.size
    rel = np.repeat(np.arange(r_), e_)
    v1 = np.stack(
        [np.bincount(inv, weights=vals * F1[c][rel], minlength=U) for c in range(2)]
    ).astype(np.float32)
    v2 = np.stack(
        [np.bincount(inv, weights=vals * F2[c][rel], minlength=U) for c in range(2)]
    ).astype(np.float32)
    ui = (uniq // N).astype(np.int64)
    uj = (uniq % N).astype(np.int64)

    def layout(part, colf, vlist):
        ptile = part >> 7
        p = part & 127
        ck = colf // CELL
        within = colf % CELL
        cellid = (ptile * 128 + p) * CH + ck
        order = np.argsort(cellid, kind="stable")
        cs = cellid[order]
        first = np.r_[True, cs[1:] != cs[:-1]] if cs.size else np.array([], bool)
        idx_first = np.maximum.accumulate(np.where(first, np.arange(cs.size), 0))
        rank = np.arange(cs.size) - idx_first
        assert cs.size == 0 or rank.max() < NI // 2, f"cell overflow: {rank.max()}"
        pt_o = ptile[order]
        p_o = p[order]
        base = ck[order] * NI + 2 * rank
        w2_ = (within[order] * 2).astype(np.int16)
        idx_arr = np.full((PT, 128, CH * NI), -1, np.int16)
        idx_arr[pt_o, p_o, base] = w2_
        idx_arr[pt_o, p_o, base + 1] = w2_ + 1
        dats = []
        for v in vlist:
            vb = np.ascontiguousarray(v[order]).view(np.uint32)
            d = np.zeros((PT, 128, CH * NI), np.uint16)
            d[pt_o, p_o, base] = (vb & 0xFFFF).astype(np.uint16)
            d[pt_o, p_o, base + 1] = (vb >> 16).astype(np.uint16)
            dats.append(d)
        return idx_arr, dats

    # A2[qn-half]: entry A[i, j] at partition i, col j - qn*HALF
    # A1T[qm-half]: entry A[i, j] at partition j, col i - qm*HALF
    a2_staged = {}
    a1t_staged = {}
    for q in range(2):
        m2 = (uj // HALF) == q
        a2_staged[q] = layout(ui[m2], uj[m2] - q * HALF, [v2[0][m2], v2[1][m2]])
        m1 = (ui // HALF) == q
        a1t_staged[q] = layout(uj[m1], ui[m1] - q * HALF, [v1[0][m1], v1[1][m1]])

    in_maps = []
    for c in range(2):
        for qm in range(2):
            for qn in range(2):
                i1, d1 = a1t_staged[qm]
                i2, d2 = a2_staged[qn]
                in_maps.append(
                    {"a1t_idx": i1, "a1t_dat": d1[c], "a2_idx": i2, "a2_dat": d2[c]}
                )
    return in_maps, F1.astype(np.float32), F2.astype(np.float32)


_NC_CACHE = {}
LAST_EXEC_TIME_NS = None


def _run_with_retry(nc, in_maps, trace, tmpdir):
    """First execution of a freshly compiled NEFF occasionally dies with
    NRT_EXEC_UNIT_UNRECOVERABLE on this setup; a re-execution succeeds."""
    last = None
    for attempt in range(3):
        try:
            return bass_utils.run_bass_kernel_spmd(
                nc, in_maps, core_ids=list(range(8)), trace=trace, tmpdir=tmpdir
            )
        except Exception as e:  # noqa: BLE001
            last = e
            sys.stderr.write(
                f"kernel: device run attempt {attempt} failed "
                f"({type(e).__name__}); retrying\n"
            )
    raise last


def kernel(edge_index, edge_value, w1, w2, num_nodes=None, trace=False, tmpdir=None):
    """Full-input entry: returns (H [2,4096,4096] f32, F1 [2,5] f32, F2 [2,5] f32)."""
    global LAST_EXEC_TIME_NS
    in_maps, F1, F2 = _host_prepare(edge_index, edge_value, w1, w2)
    if MM_DTYPE not in _NC_CACHE:
        _NC_CACHE[MM_DTYPE] = build_nc(MM_DTYPE)
    nc = _NC_CACHE[MM_DTYPE]
    if trace:
        from antenv import axon_hooks

        axon_hooks.install_default_hook()
    res = _run_with_retry(nc, in_maps, trace, tmpdir)
    LAST_EXEC_TIME_NS = res.exec_time_ns
    H = np.empty((2, N, N), np.float32)
    for c in range(2):
        for qm in range(2):
            for qn in range(2):
                blk = res.results[c * 4 + qm * 2 + qn]["h"]
                H[c, qm * HALF : (qm + 1) * HALF, qn * HALF : (qn + 1) * HALF] = blk
    return H, F1, F2
